# revision 1
# baseline (speedup 1.0000x reference)
"""AttnBlock (GroupNorm + single-head self-attention + residual) for TRN2.

8 cores = 2 batches x 4 query-chunks of 1024 tokens. Per core:
GroupNorm stats + K/V projection for the whole batch (redundant across the
4 cores of a batch, cheap vs attention) + flash attention for its queries.

v2 over v1:
  - GroupNorm affine folded into the projection weights: h = a*x + b
    (channelwise) so  w @ h = (w*a_row) @ x + (w @ b).  Projections consume
    RAW x; the serial DVE affine pass disappears; biases become tiny
    device-side matvecs.
  - j-block outer loop: K/V blocks are computed once and consumed by both
    query halves (v1 recomputed them per half).
  - PV accumulates per j-block in PSUM, then DVE-adds into SBUF
    accumulators, so PSUM stays within 8 banks.
Everything heavy runs as float32r (fp32 data, full PE rate, ~1e-4 rel err).
"""

import numpy as np
from contextlib import ExitStack

import concourse.bass as bass
import concourse.bacc as bacc
import concourse.tile as tile
from concourse import mybir
from concourse.bass_utils import run_bass_kernel_spmd

F32 = mybir.dt.float32
F32R = mybir.dt.float32r
AL = mybir.AluOpType
AF = mybir.ActivationFunctionType

B = 2
C = 512
N = 4096
NQ = 1024
P = 128
NCC = C // P      # 4
G = 32
EPS = 1e-6
NIH = NQ // 512   # 2
NJB = N // 512    # 8
SCALE = float(C) ** -0.5


def build_nc():
    nc = bacc.Bacc(None, target_bir_lowering=False)

    xf = nc.dram_tensor("xf", [C, N], F32R, kind="ExternalInput")
    xq = nc.dram_tensor("xq", [C, NQ], F32R, kind="ExternalInput")
    wall = nc.dram_tensor("wall", [C, 4 * C], F32R, kind="ExternalInput")
    cb = nc.dram_tensor("cb", [C, 3], F32, kind="ExternalInput")    # bq*s, bk, wp@bv+bp
    gaff = nc.dram_tensor("gaff", [C, 2], F32, kind="ExternalInput")
    gm = nc.dram_tensor("gm", [C, G], F32, kind="ExternalInput")    # indicator/16
    gmt = nc.dram_tensor("gmt", [G, C], F32, kind="ExternalInput")  # indicator
    out = nc.dram_tensor("out", [C, NQ], F32, kind="ExternalOutput")

    with tile.TileContext(nc) as tc, ExitStack() as ctx:
        const = ctx.enter_context(tc.tile_pool(name="const", bufs=1))
        wpool = ctx.enter_context(tc.tile_pool(name="wpool", bufs=1))
        hx = ctx.enter_context(tc.tile_pool(name="hx", bufs=1))
        qx = ctx.enter_context(tc.tile_pool(name="qx", bufs=1))
        xqp = ctx.enter_context(tc.tile_pool(name="xqp", bufs=1))
        kbp = ctx.enter_context(tc.tile_pool(name="kbp", bufs=2))
        vbp = ctx.enter_context(tc.tile_pool(name="vbp", bufs=2))
        ptp = ctx.enter_context(tc.tile_pool(name="ptp", bufs=3))
        accs = ctx.enter_context(tc.tile_pool(name="accs", bufs=1))
        tmp = ctx.enter_context(tc.tile_pool(name="tmp", bufs=2))
        scrp = ctx.enter_context(tc.tile_pool(name="scrp", bufs=1))
        mmp = ctx.enter_context(tc.tile_pool(name="mmp", bufs=3, space="PSUM"))
        pvp = ctx.enter_context(tc.tile_pool(name="pvp", bufs=1, space="PSUM"))
        lpp = ctx.enter_context(tc.tile_pool(name="lpp", bufs=1, space="PSUM"))

        # ---- tiny constant tables first (first matmuls need them) ----
        cb_sb = []
        gaff_sb = []
        gm_sb = []
        for cc in range(NCC):
            t = const.tile([P, 3], F32, tag=f"cb{cc}")
            nc.sync.dma_start(out=t[:], in_=cb[cc * P:(cc + 1) * P, :])
            cb_sb.append(t)
            t = const.tile([P, 2], F32, tag=f"ga{cc}")
            nc.sync.dma_start(out=t[:], in_=gaff[cc * P:(cc + 1) * P, :])
            gaff_sb.append(t)
            t = const.tile([P, G], F32, tag=f"gm{cc}")
            nc.sync.dma_start(out=t[:], in_=gm[cc * P:(cc + 1) * P, :])
            gm_sb.append(t)
        gmt_sb = const.tile([G, C], F32, tag="gmt")
        nc.sync.dma_start(out=gmt_sb[:], in_=gmt[:, :])
        eps_sb = const.tile([G, 1], F32, tag="eps")
        nc.vector.memset(eps_sb[:], EPS)
        ones_sb = const.tile([1, P], F32, tag="ones")
        nc.vector.memset(ones_sb[:], 1.0)
        onescol_sb = const.tile([P, 1], F32, tag="onescol")
        nc.vector.memset(onescol_sb[:], 1.0)

        # ---- x next (quarter tiles): GroupNorm stats are the critical path.
        # ACT's chunk (3) first so its slower stats start earliest; DVE's
        # chunks stream behind their DMAs.
        h_sb = {}
        NQT = N // 4
        for cc in (3, 0, 1, 2):
            for qq in range(4):
                t = hx.tile([P, NQT], F32R, tag=f"h{cc}{qq}")
                nc.sync.dma_start(
                    out=t[:],
                    in_=xf[cc * P:(cc + 1) * P, qq * NQT:(qq + 1) * NQT])
                h_sb[cc, qq] = t[:]

        # ---- weights as one wall [128, 4C] per chunk (k|v|q|p), + xq ----
        w_sb = {}
        xq_sb = []
        WIDX = {"k": 0, "v": 1, "q": 2, "p": 3}
        for cc in range(NCC):
            wt = wpool.tile([P, 4 * C], F32R, tag=f"wall{cc}")
            nc.sync.dma_start(out=wt[:], in_=wall[cc * P:(cc + 1) * P, :])
            for wname, k in WIDX.items():
                w_sb[wname, cc] = wt[:, k * C:(k + 1) * C]
        for cc in range(NCC):
            t = xqp.tile([P, NQ], F32R, tag=f"xq{cc}")
            nc.sync.dma_start(out=t[:], in_=xq[cc * P:(cc + 1) * P, :])
            xq_sb.append(t)

        # ---- GroupNorm stats -> per-channel a, b ----
        agg_ps = mmp.tile([G, 2], F32, tag="mm")
        mus = []
        for cc in range(NCC - 1):
            stats = tmp.tile([P, 8, 6], F32, tag="bst")
            for qq in range(4):
                xv = h_sb[cc, qq].bitcast(F32).rearrange(
                    "p (s f) -> p s f", f=512)
                for s in range(2):
                    nc.vector.bn_stats(out=stats[:, qq * 2 + s, :], in_=xv[:, s, :])
            mv = tmp.tile([P, 2], F32, tag="mv")
            nc.vector.bn_aggr(out=mv[:], in_=stats[:])
            mu = tmp.tile([P, 2], F32, tag=f"mu{cc}")
            nc.vector.tensor_copy(mu[:, 0:1], mv[:, 0:1])
            nc.vector.scalar_tensor_tensor(
                out=mu[:, 1:2], in0=mv[:, 0:1], scalar=mv[:, 0:1],
                in1=mv[:, 1:2], op0=AL.mult, op1=AL.add)
            mus.append(mu)
        # chunk 3 on ACT: accumulate sum(x) and sum(x^2) per quarter
        sxq = tmp.tile([P, 8], F32, tag="sxq")
        for qq in range(4):
            xh = h_sb[NCC - 1, qq].bitcast(F32)
            scr = scrp.tile([P, NQT], F32, tag="scr")
            nc.scalar.activation(out=scr[:], in_=xh, func=AF.Square,
                                 accum_out=sxq[:, 4 + qq:5 + qq])
            scr2 = scrp.tile([P, NQT], F32, tag="scr")
            nc.scalar.activation(out=scr2[:], in_=xh, func=AF.Identity,
                                 accum_out=sxq[:, qq:qq + 1])
        mu3 = tmp.tile([P, 2], F32, tag="mu3")
        t3 = tmp.tile([P, 2], F32, tag="t3")
        nc.vector.reduce_sum(out=t3[:, 0:1], in_=sxq[:, 0:4],
                             axis=mybir.AxisListType.X)
        nc.vector.reduce_sum(out=t3[:, 1:2], in_=sxq[:, 4:8],
                             axis=mybir.AxisListType.X)
        nc.vector.tensor_scalar(out=mu3[:], in0=t3[:], scalar1=1.0 / N,
                                scalar2=None, op0=AL.mult)
        mus.append(mu3)
        for cc in range(NCC):
            nc.tensor.matmul(out=agg_ps[:], lhsT=gm_sb[cc][:], rhs=mus[cc][:],
                             start=(cc == 0), stop=(cc == NCC - 1))
        eg = tmp.tile([G, 2], F32, tag="eg")
        nc.vector.tensor_copy(eg[:], agg_ps[:])
        msq = tmp.tile([G, 1], F32, tag="msq")
        nc.vector.tensor_mul(msq[:], eg[:, 0:1], eg[:, 0:1])
        grs = tmp.tile([G, 2], F32, tag="grs")
        nc.vector.tensor_copy(grs[:, 0:1], eg[:, 0:1])
        var = tmp.tile([G, 1], F32, tag="var")
        nc.vector.tensor_sub(var[:], eg[:, 1:2], msq[:])
        std = tmp.tile([G, 1], F32, tag="std")
        nc.scalar.activation(out=std[:], in_=var[:], func=AF.Sqrt, bias=eps_sb[:])
        nc.vector.reciprocal(grs[:, 1:2], std[:])

        ab_sb = []
        for cc in range(NCC):
            bc_ps = mmp.tile([P, 2], F32, tag="mm")
            nc.tensor.matmul(out=bc_ps[:],
                             lhsT=gmt_sb[:, cc * P:(cc + 1) * P], rhs=grs[:],
                             start=True, stop=True)
            ab = const.tile([P, 2], F32, tag=f"ab{cc}")
            nc.vector.tensor_mul(ab[:, 0:1], bc_ps[:, 1:2], gaff_sb[cc][:, 0:1])
            t2 = tmp.tile([P, 1], F32, tag="t2")
            nc.vector.tensor_mul(t2[:], bc_ps[:, 0:1], ab[:, 0:1])
            nc.vector.tensor_sub(ab[:, 1:2], gaff_sb[cc][:, 1:2], t2[:])
            ab_sb.append(ab)

        # ---- effective biases BEFORE scaling weights (order-safe via deps:
        # matvecs read raw-ish w? No: fold uses scaled w, so scale first) ----
        # fold a into wq/wk/wv rows (in place): w'[c,o] = w[c,o]*a[c]
        for wname in ("k", "v", "q"):
            for cc in range(NCC):
                w = w_sb[wname, cc]
                nc.vector.tensor_scalar(
                    out=w, in0=w.bitcast(F32),
                    scalar1=ab_sb[cc][:, 0:1], scalar2=None, op0=AL.mult)

        # effective biases (tiny device matvecs over b, using scaled weights):
        # q' = wq_s' @ x + (wq_s' @ b + bq_s) ; k likewise ;
        # deferred epilogue const: cpe = wp @ (wv' @ b) + (wp@bv + bp)
        bcol = tmp.tile([P, NCC], F32, tag="bcol")
        for cc in range(NCC):
            nc.vector.tensor_copy(bcol[:, cc:cc + 1], ab_sb[cc][:, 1:2])
        beff = {}
        for wname, bias_col in (("q", 0),):
            et = const.tile([P, NCC], F32, tag=f"be{wname}")
            for oc in range(NCC):
                ps = mmp.tile([P, 1], F32, tag="mm")
                for cc in range(NCC):
                    nc.tensor.matmul(
                        out=ps[:],
                        lhsT=w_sb[wname, cc][:, oc * P:(oc + 1) * P].bitcast(F32),
                        rhs=bcol[:, cc:cc + 1],
                        start=(cc == 0), stop=(cc == NCC - 1))
                nc.vector.scalar_tensor_tensor(
                    out=et[:, oc:oc + 1], in0=cb_sb[oc][:, bias_col:bias_col + 1],
                    scalar=1.0, in1=ps[:], op0=AL.mult, op1=AL.add)
            beff[wname] = et
        cpe = const.tile([P, NCC], F32, tag="cpe")

        def emit_cpe():
            wvb = tmp.tile([P, NCC], F32, tag="wvb")
            for oc in range(NCC):
                ps = mmp.tile([P, 1], F32, tag="mm")
                for cc in range(NCC):
                    nc.tensor.matmul(
                        out=ps[:],
                        lhsT=w_sb["v", cc][:, oc * P:(oc + 1) * P].bitcast(F32),
                        rhs=bcol[:, cc:cc + 1],
                        start=(cc == 0), stop=(cc == NCC - 1))
                nc.vector.tensor_copy(wvb[:, oc:oc + 1], ps[:])
            for oc in range(NCC):
                ps = mmp.tile([P, 1], F32, tag="mm")
                for cc in range(NCC):
                    nc.tensor.matmul(
                        out=ps[:],
                        lhsT=w_sb["p", cc][:, oc * P:(oc + 1) * P].bitcast(F32),
                        rhs=wvb[:, cc:cc + 1],
                        start=(cc == 0), stop=(cc == NCC - 1))
                nc.vector.scalar_tensor_tensor(
                    out=cpe[:, oc:oc + 1], in0=cb_sb[oc][:, 2:3],
                    scalar=1.0, in1=ps[:], op0=AL.mult, op1=AL.add)

        # ---- q projection from RAW xq with folded weights ----
        q_sb = []
        for oc in range(NCC):
            t = qx.tile([P, NQ], F32R, tag=f"q{oc}")
            q_sb.append(t)
        for ih in range(NIH):
            isl = slice(ih * 512, (ih + 1) * 512)
            for oc in range(NCC):
                ps = mmp.tile([P, 512], F32, tag="mm")
                for cc in range(NCC):
                    nc.tensor.matmul(
                        out=ps[:],
                        lhsT=w_sb["q", cc][:, oc * P:(oc + 1) * P],
                        rhs=xq_sb[cc][:, isl],
                        start=(cc == 0), stop=(cc == NCC - 1))
                nc.vector.tensor_scalar(
                    out=q_sb[oc][:, isl], in0=ps[:],
                    scalar1=beff["q"][:, oc:oc + 1], scalar2=None,
                    op0=AL.add)

        # ---- SBUF accumulators for attention output and l ----
        acc_sb = {}
        for ih in range(NIH):
            for cv in range(NCC):
                a_t = accs.tile([P, 512], F32R, tag=f"a{ih}{cv}")
                acc_sb[ih, cv] = a_t
        l_sb = {}
        for ih in range(NIH):
            l_t = accs.tile([1, 512], F32, tag=f"l{ih}")
            l_sb[ih] = l_t

        # ---- epilogue (per i-half), emitted inline to overlap ----
        def emit_epilogue(ih):
            isl = slice(ih * 512, (ih + 1) * 512)
            lb_ps = mmp.tile([P, 512], F32, tag="mm")
            nc.tensor.matmul(out=lb_ps[:], lhsT=ones_sb[:], rhs=l_sb[ih][:],
                             start=True, stop=True)
            rlb = tmp.tile([P, 512], F32, tag="rlb")
            nc.vector.reciprocal(rlb[:], lb_ps[:])
            for oc in range(NCC):
                ps = mmp.tile([P, 512], F32, tag="mm")
                for cv in range(NCC):
                    nc.tensor.matmul(
                        out=ps[:],
                        lhsT=w_sb["p", cv][:, oc * P:(oc + 1) * P],
                        rhs=acc_sb[ih, cv][:],
                        start=(cv == 0), stop=(cv == NCC - 1))
                fin = tmp.tile([P, 512], F32, tag="fin")
                nc.vector.tensor_mul(fin[:], ps[:], rlb[:])
                nc.vector.scalar_tensor_tensor(
                    out=fin[:], in0=fin[:], scalar=cpe[:, oc:oc + 1],
                    in1=xq_sb[oc][:, isl].bitcast(F32), op0=AL.add, op1=AL.add)
                nc.sync.dma_start(out=out[oc * P:(oc + 1) * P, isl], in_=fin[:])

        # ---- attention: j-block outer, K/V computed once ----
        for jb in range(NJB):
            if jb == 1:
                emit_cpe()
            jhsl = slice((jb % 2) * 512, (jb % 2 + 1) * 512)
            kb = []
            for oc in range(NCC):
                ps = mmp.tile([P, 512], F32, tag="mm")
                for cc in range(NCC):
                    nc.tensor.matmul(
                        out=ps[:],
                        lhsT=w_sb["k", cc][:, oc * P:(oc + 1) * P],
                        rhs=h_sb[cc, jb // 2][:, jhsl],
                        start=(cc == 0), stop=(cc == NCC - 1))
                t = kbp.tile([P, 512], F32R, tag=f"kb{oc}")
                nc.vector.tensor_copy(t[:], ps[:])
                kb.append(t)
            vb = []
            for jt in range(4):
                gh = (jb % 2) * 4 + jt
                ps = mmp.tile([P, 512], F32, tag="mm")
                for cc in range(NCC):
                    nc.tensor.matmul(
                        out=ps[:],
                        lhsT=h_sb[cc, jb // 2][:, gh * P:(gh + 1) * P],
                        rhs=w_sb["v", cc],
                        start=(cc == 0), stop=(cc == NCC - 1))
                t = vbp.tile([P, C + 1], F32R, tag=f"vb{jt}")
                nc.vector.tensor_copy(t[:, 0:C], ps[:])
                nc.vector.tensor_copy(t[:, C:C + 1], onescol_sb[:])
                vb.append(t)
            for ih in range(NIH):
                isl = slice(ih * 512, (ih + 1) * 512)
                pv_ps = []
                for cv in range(NCC):
                    pv_t = pvp.tile([P, 512], F32, tag=f"pv{cv}")
                    pv_ps.append(pv_t)
                l_ps = lpp.tile([1, 512], F32, tag="l")
                for jt in range(4):
                    ps = mmp.tile([P, 512], F32, tag="mm")
                    for oc in range(NCC):
                        nc.tensor.matmul(
                            out=ps[:],
                            lhsT=kb[oc][:, jt * P:(jt + 1) * P],
                            rhs=q_sb[oc][:, isl],
                            start=(oc == 0), stop=(oc == NCC - 1))
                    pt = ptp.tile([P, 512], F32R, tag="pt")
                    nc.scalar.activation(out=pt[:], in_=ps[:], func=AF.Exp)
                    for cv in range(NCC):
                        nc.tensor.matmul(
                            out=pv_ps[cv][:],
                            lhsT=vb[jt][:, cv * P:(cv + 1) * P],
                            rhs=pt[:],
                            start=(jt == 0), stop=(jt == 3))
                    nc.tensor.matmul(
                        out=l_ps[:], lhsT=vb[jt][:, C:C + 1], rhs=pt[:],
                        start=(jt == 0), stop=(jt == 3))
                for cv in range(NCC):
                    if jb == 0:
                        nc.vector.tensor_copy(acc_sb[ih, cv][:], pv_ps[cv][:])
                    else:
                        nc.vector.tensor_add(
                            acc_sb[ih, cv][:],
                            acc_sb[ih, cv][:].bitcast(F32), pv_ps[cv][:])
                if jb == 0:
                    nc.vector.tensor_copy(l_sb[ih][:], l_ps[:])
                else:
                    nc.vector.tensor_add(l_sb[ih][:], l_sb[ih][:], l_ps[:])
                if jb == NJB - 1 and ih == 0:
                    emit_epilogue(0)
        emit_epilogue(1)

    nc.compile()
    return nc


_NC = None


def _get_nc():
    global _NC
    if _NC is None:
        _NC = build_nc()
    return _NC


def make_in_maps(x, gn_scale, gn_bias, wq, bq, wk, bk, wv, bv, wp, bp):
    f = np.float32
    x = np.asarray(x, f)
    wq = np.asarray(wq, f); wk = np.asarray(wk, f)
    wv = np.asarray(wv, f); wp = np.asarray(wp, f)
    bq = np.asarray(bq, f); bk = np.asarray(bk, f)
    bv = np.asarray(bv, f); bp = np.asarray(bp, f)
    gn_scale = np.asarray(gn_scale, f); gn_bias = np.asarray(gn_bias, f)

    wqt = wq.T * np.float32(SCALE)
    wall = np.ascontiguousarray(
        np.concatenate([wk.T, wv.T, wqt, wp.T], axis=1), f)
    cp = wp.astype(np.float64) @ bv.astype(np.float64) + bp
    cb = np.stack([bq * np.float32(SCALE), bk, cp.astype(f)], axis=1)
    cb = np.ascontiguousarray(cb, f)
    gaff = np.ascontiguousarray(np.stack([gn_scale, gn_bias], axis=1), f)
    gmat = np.zeros((C, G), f)
    gmat[np.arange(C), np.arange(C) // (C // G)] = 1.0 / (C // G)
    gmatt = np.zeros((G, C), f)
    gmatt[np.arange(C) // (C // G), np.arange(C)] = 1.0

    in_maps = []
    for b in range(B):
        xb = np.ascontiguousarray(x[b].reshape(C, N))
        for qc in range(N // NQ):
            xqc = np.ascontiguousarray(xb[:, qc * NQ:(qc + 1) * NQ])
            in_maps.append(dict(
                xf=xb, xq=xqc, wall=wall,
                cb=cb, gaff=gaff, gm=gmat, gmt=gmatt))
    return in_maps


def assemble(results, x):
    outf = np.empty((B, C, N), np.float32)
    i = 0
    for b in range(B):
        for qc in range(N // NQ):
            outf[b, :, qc * NQ:(qc + 1) * NQ] = results[i]["out"]
            i += 1
    return outf.reshape(x.shape)


def kernel(x, gn_scale, gn_bias, wq, bq, wk, bk, wv, bv, wp, bp, **run_kwargs):
    nc = _get_nc()
    in_maps = make_in_maps(x, gn_scale, gn_bias, wq, bq, wk, bk, wv, bv, wp, bp)
    res = run_bass_kernel_spmd(nc, in_maps, core_ids=list(range(8)), **run_kwargs)
    out = assemble(res.results, np.asarray(x))
    if run_kwargs:
        return out, res
    return out



# revision 2
# speedup vs baseline: 1.6520x; 1.6520x over previous
"""AttnBlock (GroupNorm + single-head self-attention + residual) for TRN2.

8 cores = 2 batches x 4 query-chunks of 1024 tokens.

v3 math restructure ("two-matrix form"): softmax is invariant to per-query
additive constants, and the per-token 1/l commutes with the output
projection.  Folding those out, the whole block needs only two host-
precomputed CxC matrices applied to RAW x:

  scores_ij ~ u_i . x_j   (mod per-i consts), u = a*(Mqk @ h_q + cq),
      Mqk = scale * wk^T wq,  h_q = a*x_q + b (GroupNorm affine)
  A_i = sum_j exp(s_ij) x_j ,  l_i = sum_j exp(s_ij)
  out = x + (WpWv*diag(a)) @ (A/l) + [WpWv b + wp bv + bp]

So K/V/Q/P projections of the token stream disappear (6.98 -> 4.83 GMAC
per core) and the attention matmuls consume x directly in bf16 (half the
DMA, FWL weight loads).  PSUM: A 4 banks (chain over all 32 j-blocks),
scores 2, l 1, epilogue 1 = 8.
"""

import numpy as np
import ml_dtypes
from contextlib import ExitStack

import concourse.bass as bass
import concourse.bacc as bacc
import concourse.tile as tile
from concourse import mybir
from concourse.bass_utils import run_bass_kernel_spmd

F32 = mybir.dt.float32
BF16 = mybir.dt.bfloat16
AL = mybir.AluOpType
AF = mybir.ActivationFunctionType

B = 2
C = 512
N = 4096
NQ = 1024
P = 128
NCC = C // P      # 4
G = 32
EPS = 1e-6
NJB = N // P      # 32 j-blocks of 128 tokens
NIH = NQ // 512   # 2 query halves of 512
SCALE = float(C) ** -0.5
BF = ml_dtypes.bfloat16


def build_nc():
    nc = bacc.Bacc(None, target_bir_lowering=False)

    xh = nc.dram_tensor("xh", [C, N], BF16, kind="ExternalInput")
    xt = nc.dram_tensor("xt", [N, C], BF16, kind="ExternalInput")
    xq = nc.dram_tensor("xq", [C, NQ], F32, kind="ExternalInput")
    mt = nc.dram_tensor("mt", [C, C], BF16, kind="ExternalInput")    # (scale*wk^T wq)^T
    w2t = nc.dram_tensor("w2t", [C, C], BF16, kind="ExternalInput")  # (wp wv)^T
    cvec = nc.dram_tensor("cvec", [C, 2], F32, kind="ExternalInput")  # [cq, wp@bv+bp]
    gaff = nc.dram_tensor("gaff", [C, 2], F32, kind="ExternalInput")
    gm = nc.dram_tensor("gm", [C, G], F32, kind="ExternalInput")     # indicator/16
    gmt = nc.dram_tensor("gmt", [G, C], F32, kind="ExternalInput")   # indicator
    out = nc.dram_tensor("out", [C, NQ], F32, kind="ExternalOutput")

    with tile.TileContext(nc) as tc, ExitStack() as ctx:
        const = ctx.enter_context(tc.tile_pool(name="const", bufs=1))
        xhp = ctx.enter_context(tc.tile_pool(name="xhp", bufs=1))
        xtp = ctx.enter_context(tc.tile_pool(name="xtp", bufs=1))
        xqp = ctx.enter_context(tc.tile_pool(name="xqp", bufs=1))
        wp_ = ctx.enter_context(tc.tile_pool(name="wp", bufs=1))
        utp = ctx.enter_context(tc.tile_pool(name="utp", bufs=1))
        ptp = ctx.enter_context(tc.tile_pool(name="ptp", bufs=3))
        alp = ctx.enter_context(tc.tile_pool(name="alp", bufs=1))
        tmp = ctx.enter_context(tc.tile_pool(name="tmp", bufs=2))
        psA = ctx.enter_context(tc.tile_pool(name="psA", bufs=1, space="PSUM"))
        psS = ctx.enter_context(tc.tile_pool(name="psS", bufs=2, space="PSUM"))
        psL = ctx.enter_context(tc.tile_pool(name="psL", bufs=1, space="PSUM"))
        psE = ctx.enter_context(tc.tile_pool(name="psE", bufs=1, space="PSUM"))

        # ---- tiny constant tables ----
        cvec_sb = []
        gaff_sb = []
        gm_sb = []
        for cc in range(NCC):
            t = const.tile([P, 2], F32, tag=f"cv{cc}", name=f"cv{cc}")
            nc.sync.dma_start(out=t[:], in_=cvec[cc * P:(cc + 1) * P, :])
            cvec_sb.append(t)
            t = const.tile([P, 2], F32, tag=f"ga{cc}", name=f"ga{cc}")
            nc.sync.dma_start(out=t[:], in_=gaff[cc * P:(cc + 1) * P, :])
            gaff_sb.append(t)
            t = const.tile([P, G], F32, tag=f"gm{cc}", name=f"gm{cc}")
            nc.sync.dma_start(out=t[:], in_=gm[cc * P:(cc + 1) * P, :])
            gm_sb.append(t)
        gmt_sb = const.tile([G, C], F32, tag="gmt")
        nc.sync.dma_start(out=gmt_sb[:], in_=gmt[:, :])
        eps_sb = const.tile([G, 1], F32, tag="eps")
        nc.vector.memset(eps_sb[:], EPS)
        ones_row = const.tile([1, P], F32, tag="onesr")
        nc.vector.memset(ones_row[:], 1.0)
        ones_col = const.tile([P, 1], BF16, tag="onesc")
        nc.vector.memset(ones_col[:], 1.0)

        # ---- x (bf16) quarter tiles: GroupNorm stats stream behind DMA ----
        NQT = N // 4
        xh_sb = {}
        for cc in range(NCC):
            for qq in range(4):
                t = xhp.tile([P, NQT], BF16, tag=f"xh{cc}{qq}", name=f"xh{cc}{qq}")
                nc.sync.dma_start(
                    out=t[:],
                    in_=xh[cc * P:(cc + 1) * P, qq * NQT:(qq + 1) * NQT])
                xh_sb[cc, qq] = t

        # ---- xq (fp32), u-matrix, then lower-priority w2t / xt ----
        xq_sb = []
        for cc in range(NCC):
            t = xqp.tile([P, NQ], F32, tag=f"xq{cc}", name=f"xq{cc}")
            nc.sync.dma_start(out=t[:], in_=xq[cc * P:(cc + 1) * P, :])
            xq_sb.append(t)
        mt_sb = []
        for cc in range(NCC):
            t = wp_.tile([P, C], BF16, tag=f"mt{cc}", name=f"mt{cc}")
            nc.sync.dma_start(out=t[:], in_=mt[cc * P:(cc + 1) * P, :])
            mt_sb.append(t)
        w2t_sb = []
        for cc in range(NCC):
            t = wp_.tile([P, C], BF16, tag=f"w2t{cc}", name=f"w2t{cc}")
            nc.sync.dma_start(out=t[:], in_=w2t[cc * P:(cc + 1) * P, :])
            w2t_sb.append(t)
        xt_sb = []
        for jb in range(NJB):
            t = xtp.tile([P, C], BF16, tag=f"xt{jb}", name=f"xt{jb}")
            nc.sync.dma_start(out=t[:], in_=xt[jb * P:(jb + 1) * P, :])
            xt_sb.append(t)

        # ---- GroupNorm stats -> per-channel a, b ----
        mus = []
        for cc in range(NCC):
            stats = tmp.tile([P, 8, 6], F32, tag="bst")
            for qq in range(4):
                xv = xh_sb[cc, qq].rearrange("p (s f) -> p s f", f=512)
                for s in range(2):
                    nc.vector.bn_stats(out=stats[:, qq * 2 + s, :], in_=xv[:, s, :])
            mv = tmp.tile([P, 2], F32, tag="mv")
            nc.vector.bn_aggr(out=mv[:], in_=stats[:])
            mu = tmp.tile([P, 2], F32, tag=f"mu{cc}")
            nc.vector.tensor_copy(mu[:, 0:1], mv[:, 0:1])
            nc.vector.scalar_tensor_tensor(
                out=mu[:, 1:2], in0=mv[:, 0:1], scalar=mv[:, 0:1],
                in1=mv[:, 1:2], op0=AL.mult, op1=AL.add)
            mus.append(mu)
        agg_ps = psE.tile([G, 2], F32, tag="e")
        for cc in range(NCC):
            nc.tensor.matmul(out=agg_ps[:], lhsT=gm_sb[cc][:], rhs=mus[cc][:],
                             start=(cc == 0), stop=(cc == NCC - 1))
        eg = tmp.tile([G, 2], F32, tag="eg")
        nc.vector.tensor_copy(eg[:], agg_ps[:])
        msq = tmp.tile([G, 1], F32, tag="msq")
        nc.vector.tensor_mul(msq[:], eg[:, 0:1], eg[:, 0:1])
        grs = tmp.tile([G, 2], F32, tag="grs")
        nc.vector.tensor_copy(grs[:, 0:1], eg[:, 0:1])
        var = tmp.tile([G, 1], F32, tag="var")
        nc.vector.tensor_sub(var[:], eg[:, 1:2], msq[:])
        std = tmp.tile([G, 1], F32, tag="std")
        nc.scalar.activation(out=std[:], in_=var[:], func=AF.Sqrt, bias=eps_sb[:])
        nc.vector.reciprocal(grs[:, 1:2], std[:])

        ab_sb = []
        for cc in range(NCC):
            bc_ps = psE.tile([P, 2], F32, tag="e")
            nc.tensor.matmul(out=bc_ps[:],
                             lhsT=gmt_sb[:, cc * P:(cc + 1) * P], rhs=grs[:],
                             start=True, stop=True)
            ab = const.tile([P, 2], F32, tag=f"ab{cc}", name=f"ab{cc}")
            nc.vector.tensor_mul(ab[:, 0:1], bc_ps[:, 1:2], gaff_sb[cc][:, 0:1])
            t2 = tmp.tile([P, 1], F32, tag="t2")
            nc.vector.tensor_mul(t2[:], bc_ps[:, 0:1], ab[:, 0:1])
            nc.vector.tensor_sub(ab[:, 1:2], gaff_sb[cc][:, 1:2], t2[:])
            ab_sb.append(ab)

        # ---- h_q = a*x_q + b  (bf16) ----
        hq_sb = []
        for cc in range(NCC):
            t = utp.tile([P, NQ], BF16, tag=f"hq{cc}", name=f"hq{cc}")
            nc.vector.tensor_scalar(
                out=t[:], in0=xq_sb[cc][:],
                scalar1=ab_sb[cc][:, 0:1], scalar2=ab_sb[cc][:, 1:2],
                op0=AL.mult, op1=AL.add)
            hq_sb.append(t)

        # ---- u = a*(Mqk @ h_q + cq)  (bf16) ----
        ut_sb = []
        for oc in range(NCC):
            t = utp.tile([P, NQ], BF16, tag=f"ut{oc}", name=f"ut{oc}")
            ut_sb.append(t)
        for ih in range(NIH):
            isl = slice(ih * 512, (ih + 1) * 512)
            for oc in range(NCC):
                ups = psS.tile([P, 512], F32, tag="s")
                for cc in range(NCC):
                    nc.tensor.matmul(
                        out=ups[:],
                        lhsT=mt_sb[cc][:, oc * P:(oc + 1) * P],
                        rhs=hq_sb[cc][:, isl],
                        start=(cc == 0), stop=(cc == NCC - 1))
                nc.vector.tensor_scalar(
                    out=ut_sb[oc][:, isl], in0=ups[:],
                    scalar1=cvec_sb[oc][:, 0:1], scalar2=ab_sb[oc][:, 0:1],
                    op0=AL.add, op1=AL.mult)

        # ---- deferred consts: cb2 = W2 @ b + cpv, then scale w2t by a ----
        cb2 = const.tile([P, NCC], F32, tag="cb2")

        def emit_cb2():
            bcol = const.tile([P, NCC], BF16, tag="bcol")
            for cc in range(NCC):
                nc.vector.tensor_copy(bcol[:, cc:cc + 1], ab_sb[cc][:, 1:2])
            for oc in range(NCC):
                ps = psE.tile([P, 1], F32, tag="e")
                for cc in range(NCC):
                    nc.tensor.matmul(
                        out=ps[:],
                        lhsT=w2t_sb[cc][:, oc * P:(oc + 1) * P],
                        rhs=bcol[:, cc:cc + 1],
                        start=(cc == 0), stop=(cc == NCC - 1))
                nc.vector.scalar_tensor_tensor(
                    out=cb2[:, oc:oc + 1], in0=cvec_sb[oc][:, 1:2],
                    scalar=1.0, in1=ps[:], op0=AL.mult, op1=AL.add)

        def emit_w2scale():
            for cc in range(NCC):
                nc.vector.tensor_scalar(
                    out=w2t_sb[cc][:], in0=w2t_sb[cc][:],
                    scalar1=ab_sb[cc][:, 0:1], scalar2=None, op0=AL.mult)

        # ---- attention j-loop, software-pipelined scores ----
        pts = {}

        def emit_S(ih, jb):
            isl = slice(ih * 512, (ih + 1) * 512)
            qq, jo = jb // 8, (jb % 8) * P
            S = psS.tile([P, 512], F32, tag="s")
            for cc in range(NCC):
                nc.tensor.matmul(
                    out=S[:],
                    lhsT=xh_sb[cc, qq][:, jo:jo + P],
                    rhs=ut_sb[cc][:, isl],
                    start=(cc == 0), stop=(cc == NCC - 1))
            pt = ptp.tile([P, 512], BF16, tag="pt")
            nc.scalar.activation(out=pt[:], in_=S[:], func=AF.Exp)
            pts[ih, jb] = pt

        def emit_LA(ih, jb, A, lp):
            pt = pts.pop((ih, jb))
            nc.tensor.matmul(out=lp[:], lhsT=ones_col[:], rhs=pt[:],
                             start=(jb == 0), stop=(jb == NJB - 1))
            for cv in range(NCC):
                nc.tensor.matmul(
                    out=A[cv][:],
                    lhsT=xt_sb[jb][:, cv * P:(cv + 1) * P],
                    rhs=pt[:],
                    start=(jb == 0), stop=(jb == NJB - 1))

        def emit_epilogue(ih, A, lp, interleave=()):
            isl = slice(ih * 512, (ih + 1) * 512)
            for thunk in interleave:
                thunk()
            lsb = tmp.tile([1, 512], F32, tag="lsb")
            nc.vector.tensor_copy(lsb[:], lp[:])
            lb_ps = psE.tile([P, 512], F32, tag="e")
            nc.tensor.matmul(out=lb_ps[:], lhsT=ones_row[:], rhs=lsb[:],
                             start=True, stop=True)
            rlb = tmp.tile([P, 512], F32, tag="rlb")
            nc.vector.reciprocal(rlb[:], lb_ps[:])
            Al = []
            for cv in range(NCC):
                t = alp.tile([P, 512], BF16, tag=f"al{cv}", name=f"al{cv}")
                nc.vector.tensor_mul(t[:], A[cv][:], rlb[:])
                Al.append(t)
            for oc in range(NCC):
                fps = psE.tile([P, 512], F32, tag="e")
                for cc in range(NCC):
                    nc.tensor.matmul(
                        out=fps[:],
                        lhsT=w2t_sb[cc][:, oc * P:(oc + 1) * P],
                        rhs=Al[cc][:],
                        start=(cc == 0), stop=(cc == NCC - 1))
                fin = tmp.tile([P, 512], F32, tag="fin")
                nc.vector.scalar_tensor_tensor(
                    out=fin[:], in0=fps[:], scalar=cb2[:, oc:oc + 1],
                    in1=xq_sb[oc][:, isl], op0=AL.add, op1=AL.add)
                nc.sync.dma_start(out=out[oc * P:(oc + 1) * P, isl], in_=fin[:])

        def alloc_acc(ih):
            A = []
            for cv in range(NCC):
                t = psA.tile([P, 512], F32, tag=f"a{cv}", name=f"a{cv}")
                A.append(t)
            lp = psL.tile([1, 512], F32, tag="l")
            return A, lp

        A0, lp0 = alloc_acc(0)
        emit_S(0, 0)
        for jb in range(NJB):
            if jb + 1 < NJB:
                emit_S(0, jb + 1)
            emit_LA(0, jb, A0, lp0)
            if jb == 1:
                emit_cb2()
            if jb == 3:
                emit_w2scale()
        A1, lp1 = alloc_acc(1)
        emit_epilogue(0, A0, lp0,
                      interleave=(lambda: emit_S(1, 0), lambda: emit_S(1, 1)))
        for jb in range(NJB):
            if jb + 1 < NJB and (1, jb + 1) not in pts:
                emit_S(1, jb + 1)
            emit_LA(1, jb, A1, lp1)
        emit_epilogue(1, A1, lp1)

    nc.compile()
    return nc


_NC = None


def _get_nc():
    global _NC
    if _NC is None:
        _NC = build_nc()
    return _NC


def make_in_maps(x, gn_scale, gn_bias, wq, bq, wk, bk, wv, bv, wp, bp):
    f = np.float32
    d = np.float64
    x = np.asarray(x, f)
    wq = np.asarray(wq, f); wk = np.asarray(wk, f)
    wv = np.asarray(wv, f); wp = np.asarray(wp, f)
    bq = np.asarray(bq, f); bk = np.asarray(bk, f)
    bv = np.asarray(bv, f); bp = np.asarray(bp, f)
    gn_scale = np.asarray(gn_scale, f); gn_bias = np.asarray(gn_bias, f)

    # lhsT for u-projection: (Mqk)^T = scale * wq^T wk
    mt_np = np.ascontiguousarray(
        (SCALE * (wq.T.astype(d) @ wk.astype(d))).astype(f)).astype(BF)
    # lhsT for output projection: (wp wv)^T
    w2t_np = np.ascontiguousarray(
        (wp.astype(d) @ wv.astype(d)).T.astype(f)).astype(BF)
    cq_np = (SCALE * (wk.T.astype(d) @ bq.astype(d))).astype(f)
    cpv_np = (wp.astype(d) @ bv.astype(d) + bp).astype(f)
    cvec = np.ascontiguousarray(np.stack([cq_np, cpv_np], axis=1), f)
    gaff = np.ascontiguousarray(np.stack([gn_scale, gn_bias], axis=1), f)
    gmat = np.zeros((C, G), f)
    gmat[np.arange(C), np.arange(C) // (C // G)] = 1.0 / (C // G)
    gmatt = np.zeros((G, C), f)
    gmatt[np.arange(C) // (C // G), np.arange(C)] = 1.0

    in_maps = []
    for b in range(B):
        xb = np.ascontiguousarray(x[b].reshape(C, N))
        xh_b = xb.astype(BF)
        xt_b = np.ascontiguousarray(xb.T).astype(BF)
        for qc in range(N // NQ):
            xqc = np.ascontiguousarray(xb[:, qc * NQ:(qc + 1) * NQ])
            in_maps.append(dict(
                xh=xh_b, xt=xt_b, xq=xqc, mt=mt_np, w2t=w2t_np,
                cvec=cvec, gaff=gaff, gm=gmat, gmt=gmatt))
    return in_maps


def assemble(results, x):
    outf = np.empty((B, C, N), np.float32)
    i = 0
    for b in range(B):
        for qc in range(N // NQ):
            outf[b, :, qc * NQ:(qc + 1) * NQ] = results[i]["out"]
            i += 1
    return outf.reshape(x.shape)


def kernel(x, gn_scale, gn_bias, wq, bq, wk, bk, wv, bv, wp, bp, **run_kwargs):
    nc = _get_nc()
    in_maps = make_in_maps(x, gn_scale, gn_bias, wq, bq, wk, bk, wv, bv, wp, bp)
    res = run_bass_kernel_spmd(nc, in_maps, core_ids=list(range(8)), **run_kwargs)
    out = assemble(res.results, np.asarray(x))
    if run_kwargs:
        return out, res
    return out
